# revision 1
# baseline (speedup 1.0000x reference)
"""GQA attention layer (QKV proj + RoPE + causal GQA attention + o_proj) on 8 trn2
NeuronCores.

Sharding: DP=2 over batch x TP=4 over heads (8 Q heads / 2 KV heads per core).

All-bf16 data path (x, Wq/Wk/Wv/Wo, q/k/v, p, attention out; f32 PSUM + f32
RoPE tables/biases).  Emission is software-pipelined across engines: the QKV
projection of block sb+1 is emitted as filler between the attention kt-steps of
block sb, so the PE has independent work during the scores->exp->mask->AV
round-trips; the o_proj of gathered blocks fills the last block's attention.
Attention uses transposed scores (exp on ACT, bf16 p, ones-matmul softmax sums,
binary causal masks post-exp, per-tile causal column trimming), per-qs-block
bf16 AllGather (last block gathered per d-tile) overlapped with compute, and a
column-sharded o_proj with d-major accumulation in the tail.  Host reassembles
[B, S, H] from per-core column shards.
"""

import ml_dtypes
import numpy as np

import concourse.bass as bass
import concourse.bacc as bacc
import concourse.tile as tile
import concourse.mybir as mybir
from concourse import bass_utils

F32 = mybir.dt.float32
BF16 = mybir.dt.bfloat16

# Model shape (hardcoded for nn_Attention_38147899523668)
B, S, H = 2, 2048, 2048
NH, NKV, HD = 32, 8, 64
G = NH // NKV
SCALE = 1.0 / np.sqrt(HD)

# Sharding
N_CORES = 8
TP = 4                    # tensor-parallel group size (heads)
DP = N_CORES // TP        # data-parallel over batch
NH_L = NH // TP           # 8 Q heads per core
NKV_L = NKV // TP         # 2 KV heads per core
DQ = NH_L * HD            # 512 rows of q per core
DKV = NKV_L * HD          # 128 rows of k/v per core

SBLK = 512                # qs block (also matmul N)
SWAPMASK = [i ^ 1 for i in range(32)]   # adjacent-pair swap (RoPE interleaved)
KT = 128                  # ks tile
N_SB = S // SBLK
N_HT = H // 128
N_DT = DQ // 128
N_ST = S // 128
N_FT = TP * N_DT
N_DIAG = SBLK // KT


def build_program(nc, tp_groups=None, fake_gather=False):
    if tp_groups is None:
        tp_groups = [[0, 1, 2, 3], [4, 5, 6, 7]]
    tp = len(tp_groups[0])
    s = S
    EO = DQ

    inp = {}
    def din(name, shape, dtype=F32):
        inp[name] = nc.dram_tensor(name, shape, dtype, kind="ExternalInput").ap()
        return inp[name]

    xT = din("xT", [H, s], dtype=BF16)
    wqT = din("wqT", [H, DQ], dtype=BF16)
    wkT = din("wkT", [H, DKV], dtype=BF16)
    wvT = din("wvT", [H, DKV], dtype=BF16)
    bq = din("bq", [128, N_DT])
    bk = din("bk", [128, 1])
    bv = din("bv", [128, 1])
    cos_rep = din("cos_rep", [128, s])
    sin_pm = din("sin_pm", [128, s])
    woT = din("woT", [H, EO], dtype=BF16)
    out = nc.dram_tensor("out", [s, EO], F32, kind="ExternalOutput").ap()

    with tile.TileContext(nc) as tc:
        with tc.tile_pool(name="dram", bufs=1, space="DRAM") as dram, \
             tc.tile_pool(name="psP", bufs=2, space="PSUM") as psP, \
             tc.tile_pool(name="psS", bufs=2, space="PSUM") as psS, \
             tc.tile_pool(name="psC", bufs=4, space="PSUM") as psC, \
             tc.tile_pool(name="acts", bufs=1) as acts, \
             tc.tile_pool(name="consts", bufs=1) as consts, \
             tc.tile_pool(name="w1", bufs=1) as w1, \
             tc.tile_pool(name="xs", bufs=6) as xs, \
             tc.tile_pool(name="rtmp", bufs=2) as rtmp, \
             tc.tile_pool(name="pT", bufs=8) as pT_p, \
             tc.tile_pool(name="nrm", bufs=2) as nrm_p, \
             tc.tile_pool(name="afl", bufs=2) as afl_p, \
             tc.tile_pool(name="af3", bufs=1) as af3_p, \
             tc.tile_pool(name="osb", bufs=3) as osb_p:

            # AllGather staging: per qs block; last block staged per d-tile
            ag_in = [dram.tile([N_DT, 128, SBLK], BF16, tag=f"agi{b_}",
                               name=f"agi{b_}") for b_ in range(N_SB - 1)]
            ag3_in = [[dram.tile([128, SBLK // 2], BF16, tag=f"ag3i{d}_{h2}",
                                 name=f"ag3i{d}_{h2}") for h2 in range(2)]
                      for d in range(N_DT)]
            ag_out = [dram.tile([tp, N_DT, 128, SBLK], BF16, tag=f"ago{b_}",
                                name=f"ago{b_}") for b_ in range(N_SB - 1)]
            ag3_out = [[dram.tile([tp, 128, SBLK // 2], BF16,
                                  tag=f"ag3o{d}_{h2}", name=f"ag3o{d}_{h2}")
                        for h2 in range(2)] for d in range(N_DT)]

            qrot = acts.tile([128, N_DT, s], BF16, tag="qrot")
            kT_rep = acts.tile([128, NKV_L, s], BF16, tag="kT_rep")
            v_sb = acts.tile([128, N_ST, NKV_L * HD], BF16, tag="v_sb")

            ones16 = consts.tile([128, 64], BF16, tag="ones16")
            nc.vector.memset(ones16[:], 1.0)
            cos_sb = consts.tile([128, s], F32, tag="cos")
            sin_sb = consts.tile([128, s], F32, tag="sin")
            bq_sb = consts.tile([128, N_DT], F32, tag="bq")
            bk_sb = consts.tile([128, 1], F32, tag="bk")
            bv_sb = consts.tile([128, 1], F32, tag="bv")

            # binary causal mask for the diagonal ks tiles of a qs block:
            # maskc[p, m, q] = 1.0 if q >= KT*m + p else 0.0
            maskc = consts.tile([128, N_DIAG, SBLK], BF16, tag="maskc")
            nc.gpsimd.memset(maskc[:], 1.0)
            nc.gpsimd.affine_select(
                out=maskc[:], in_=maskc[:],
                compare_op=mybir.AluOpType.is_ge,
                fill=0.0, base=0,
                pattern=[[-KT, N_DIAG], [1, SBLK]],
                channel_multiplier=-1,
            )

            ident = consts.tile([128, 128], F32, tag="ident")
            nc.gpsimd.memset(ident[:], 0.0)
            nc.gpsimd.affine_select(
                out=ident[:], in_=ident[:],
                compare_op=mybir.AluOpType.not_equal,
                fill=1.0, base=0,
                pattern=[[-1, 128]], channel_multiplier=1,
            )

            # ---- weight + const DMAs (scalar queue; idle at startup) ----
            wq_sb = w1.tile([128, N_HT, DQ], BF16, tag="wq")
            wk_sb = w1.tile([128, N_HT, DKV], BF16, tag="wk")
            wv_sb = w1.tile([128, N_HT, DKV], BF16, tag="wv")
            woT_sb = w1.tile([128, N_HT, EO], BF16, tag="wo")
            wq_c = wqT.rearrange("(c g p) d -> c p g d", p=128, g=4)
            wk_t = wkT.rearrange("(t p) d -> t p d", p=128)
            wv_t = wvT.rearrange("(t p) d -> t p d", p=128)
            wo_c = woT.rearrange("(c g p) d -> c p g d", p=128, g=4)
            nc.scalar.dma_start(out=wq_sb[:, 0:1, :], in_=wq_c[0][:, 0:1, :])
            nc.scalar.dma_start(out=wq_sb[:, 1:4, :], in_=wq_c[0][:, 1:4, :])
            nc.scalar.dma_start(out=wk_sb[:, 0, :], in_=wk_t[0])
            nc.scalar.dma_start(out=wv_sb[:, 0, :], in_=wv_t[0])
            nc.scalar.dma_start(out=bq_sb[:], in_=bq[:])
            nc.scalar.dma_start(out=bk_sb[:], in_=bk[:])
            nc.scalar.dma_start(out=bv_sb[:], in_=bv[:])
            for c_ in range(1, N_HT // 4):
                nc.scalar.dma_start(out=wq_sb[:, 4 * c_:4 * (c_ + 1), :],
                                    in_=wq_c[c_])
                for t_ in range(4 * c_ - 3, 4 * c_ + 1):
                    nc.scalar.dma_start(out=wk_sb[:, t_, :], in_=wk_t[t_])
            for t_ in range(13, N_HT):
                nc.scalar.dma_start(out=wk_sb[:, t_, :], in_=wk_t[t_])
            nc.scalar.dma_start(out=cos_sb[:], in_=cos_rep[:])
            nc.scalar.dma_start(out=sin_sb[:], in_=sin_pm[:])
            for c_ in range(N_HT // 4):
                nc.scalar.dma_start(out=woT_sb[:, 4 * c_:4 * (c_ + 1), :],
                                    in_=wo_c[c_])

            xT_c = xT.rearrange("(c g p) s -> c p g s", p=128, g=4)
            xt_tiles = {}

            def load_x_block(sb, split_first=False):
                ss = sb * SBLK
                for hc in range(N_HT // 4):
                    t = xs.tile([128, 4, SBLK], BF16, tag="xt", name="xt")
                    if split_first and hc == 0:
                        nc.sync.dma_start(out=t[:, 0:1, :],
                                          in_=xT_c[hc, :, 0:1, ss:ss + SBLK])
                        nc.sync.dma_start(out=t[:, 1:4, :],
                                          in_=xT_c[hc, :, 1:4, ss:ss + SBLK])
                    else:
                        nc.sync.dma_start(out=t[:],
                                          in_=xT_c[hc, :, :, ss:ss + SBLK])
                    if split_first:
                        # stream the wv tails between the x tiles so each
                        # hc's weights land just before its matmuls
                        for t_ in range(4 * hc + 1, 4 * hc + 5):
                            if t_ < N_HT:
                                nc.sync.dma_start(out=wv_sb[:, t_, :],
                                                  in_=wv_t[t_])
                    xt_tiles[(sb, hc)] = t

            # ---------------- epilogue op makers (DVE/PE) ----------------
            def q_epilogue_ops(sb, dt, psq_t):
                ss = sb * SBLK
                ops = []
                st = {}
                def add():
                    st["qraw"] = rtmp.tile([128, SBLK], F32, tag="raw", name="qraw")
                    nc.vector.tensor_scalar_add(
                        out=st["qraw"][:], in0=psq_t[:],
                        scalar1=bq_sb[:, dt:dt + 1])
                def shuf():
                    st["qsw"] = rtmp.tile([128, SBLK], F32, tag="sw", name="qsw")
                    nc.vector.stream_shuffle(st["qsw"][:], st["qraw"][:], SWAPMASK)
                def mul1():
                    st["qcos"] = rtmp.tile([128, SBLK], F32, tag="cm", name="qcos")
                    nc.vector.tensor_mul(st["qcos"][:], st["qraw"][:],
                                         cos_sb[:, ss:ss + SBLK])
                def mul2():
                    nc.vector.tensor_mul(st["qsw"][:], st["qsw"][:],
                                         sin_sb[:, ss:ss + SBLK])
                def add2():
                    nc.vector.tensor_add(qrot[:, dt, ss:ss + SBLK],
                                         st["qcos"][:], st["qsw"][:])
                ops += [add, shuf, mul1, mul2, add2]
                return ops

            def k_epilogue_ops(sb, psk_t):
                ss = sb * SBLK
                ops = []
                st = {}
                def add():
                    st["kraw"] = rtmp.tile([128, SBLK], F32, tag="raw", name="kraw")
                    nc.vector.tensor_scalar_add(
                        out=st["kraw"][:], in0=psk_t[:], scalar1=bk_sb[:, 0:1])
                def shuf():
                    st["ksw"] = rtmp.tile([128, SBLK], F32, tag="sw", name="ksw")
                    nc.vector.stream_shuffle(st["ksw"][:], st["kraw"][:], SWAPMASK)
                def mul1():
                    st["kcos"] = rtmp.tile([128, SBLK], F32, tag="cm", name="kcos")
                    nc.vector.tensor_mul(st["kcos"][:], st["kraw"][:],
                                         cos_sb[:, ss:ss + SBLK])
                def mul2():
                    nc.vector.tensor_mul(st["ksw"][:], st["ksw"][:],
                                         sin_sb[:, ss:ss + SBLK])
                def add2():
                    st["krot"] = rtmp.tile([128, SBLK], BF16, tag="krot", name="krot")
                    nc.vector.tensor_add(st["krot"][:], st["kcos"][:], st["ksw"][:])
                def stores():
                    for j in range(NKV_L):
                        for half in range(2):
                            nc.sync.dma_start(
                                out=kT_rep[half * 64:half * 64 + 64, j,
                                           ss:ss + SBLK],
                                in_=st["krot"][j * 64:j * 64 + 64, :])
                ops += [add, shuf, mul1, mul2, add2, stores]
                return ops

            def v_epilogue_ops(sb, psv_t):
                ops = []
                st = {}
                def add():
                    st["vtb"] = rtmp.tile([128, SBLK], F32, tag="vtb", name="vtb")
                    nc.vector.tensor_scalar_add(
                        out=st["vtb"][:], in0=psv_t[:], scalar1=bv_sb[:, 0:1])
                ops.append(add)
                for k in range(SBLK // 128):
                    stt = sb * (SBLK // 128) + k
                    def tpose(k=k, stt=stt):
                        ptp = psP.tile([128, 128], F32, tag="pP",
                                       name="ptp",
                                       padded_shape=[128, SBLK])
                        nc.tensor.transpose(
                            ptp[:], st["vtb"][:, k * 128:(k + 1) * 128], ident[:])
                        nc.vector.tensor_copy(v_sb[:, stt, :], ptp[:])
                    ops.append(tpose)
                return ops

            # LazyPs: defer psum-tile lookup until the closure runs
            class LazyPs:
                def __init__(self, st):
                    self.st = st
                def __getitem__(self, idx):
                    return self.st["ps"][idx]

            # ---------------- QKV pass-structure (blocks >= 1) ----------------
            def qkv_fillers(sb):
                """Closure list: 6 single-PSUM-bank passes + epilogues."""
                ops = []
                passes = [("q", 0), ("q", 1), ("q", 2), ("q", 3),
                          ("k", None), ("v", None)]
                for kind, dt in passes:
                    st = {}
                    lazy = LazyPs(st)
                    def alloc(st=st):
                        st["ps"] = psP.tile([128, SBLK], F32, tag="pP", name="psqkv")
                    ops.append(alloc)
                    for hc in range(N_HT // 4):
                        def mms(kind=kind, dt=dt, hc=hc, st=st, sb=sb):
                            xt = xt_tiles[(sb, hc)]
                            for hg in range(4):
                                ht = 4 * hc + hg
                                if kind == "q":
                                    w_ap = wq_sb[:, ht, dt * 128:(dt + 1) * 128]
                                elif kind == "k":
                                    w_ap = wk_sb[:, ht, :]
                                else:
                                    w_ap = wv_sb[:, ht, :]
                                nc.tensor.matmul(
                                    st["ps"][:], w_ap, xt[:, hg, :],
                                    start=(ht == 0), stop=(ht == N_HT - 1),
                                    skip_group_check=True)
                        ops.append(mms)
                    if kind == "q":
                        ops += q_epilogue_ops(sb, dt, lazy)
                    elif kind == "k":
                        ops += k_epilogue_ops(sb, lazy)
                    else:
                        ops += v_epilogue_ops(sb, lazy)
                return ops

            # ---------------- o_proj for one gathered qs block ----------------
            def oproj_fillers(bk_, gathered_ap):
                """(dma_closure, matmul closures) for one gathered qs block."""
                ops = []
                st = {}
                def dma_afull():
                    st["af"] = afl_p.tile([128, N_FT, SBLK], BF16, tag="afull", name="afull")
                    nc.scalar.dma_start(
                        out=st["af"][:],
                        in_=gathered_ap.rearrange("f p s -> p f s"))
                for sti_l in range(SBLK // 128):
                    sti = bk_ * (SBLK // 128) + sti_l
                    pst = {}
                    def alloc(pst=pst):
                        pst["pso"] = psP.tile([128, EO], F32, tag="pP", name="pso")
                    ops.append(alloc)
                    for fg in range(4):
                        def mms(sti_l=sti_l, fg=fg, st=st, pst=pst):
                            for fi in range(4):
                                ft = 4 * fg + fi
                                nc.tensor.matmul(
                                    pst["pso"][:],
                                    st["af"][:, ft,
                                             sti_l * 128:(sti_l + 1) * 128],
                                    woT_sb[:, ft, :],
                                    start=(ft == 0), stop=(ft == N_FT - 1),
                                    skip_group_check=True)
                        ops.append(mms)
                    def fin(sti=sti, pst=pst):
                        ot = osb_p.tile([128, EO], F32, tag="ot", name="ot")
                        nc.scalar.activation(
                            out=ot[:], in_=pst["pso"][:],
                            func=mybir.ActivationFunctionType.Copy, scale=1.0)
                        nc.scalar.dma_start(
                            out=out[sti * 128:(sti + 1) * 128, :], in_=ot[:])
                    ops.append(fin)
                return dma_afull, ops

            # ---------------- attention for one qs block ----------------
            def emit_attention(qi, fillers, pre_pump=0, post_strip=None,
                               pump_strips=None):
                """Emit attention strips for block qi, pumping filler closures
                between kt-steps."""
                qs = qi * SBLK
                nk = (qi + 1) * N_DIAG
                dt0 = nk - N_DIAG
                nsteps = (pump_strips if pump_strips is not None
                          else NKV_L * 2) * nk
                it = iter(fillers)
                for _ in range(pre_pump):
                    op = next(it, None)
                    if op is not None:
                        op()
                acc = [0.0]
                per = max(len(fillers) - pre_pump, 0) / max(nsteps, 1)
                def pump():
                    acc[0] += per
                    while acc[0] >= 1.0:
                        op = next(it, None)
                        if op is None:
                            return
                        op()
                        acc[0] -= 1.0
                for j in range(NKV_L):
                    for pr in range(2):
                        dt = 2 * j + pr
                        ctx = psC.tile([128, SBLK], F32, tag="pC")
                        sums = psC.tile([128, SBLK], F32, tag="pC")
                        for kt in range(nk):
                            m = kt - dt0
                            off = KT * m if m >= 1 else 0
                            ks = kt * KT
                            first, last = kt == 0, kt == nk - 1
                            sc = [psS.tile([128, SBLK], F32, tag="pS",
                                           name=f"sc{h_}") for h_ in range(2)]
                            for h in range(2):
                                hb = h * 64
                                nc.tensor.matmul(
                                    sc[h][:, off:],
                                    kT_rep[hb:hb + 64, j, ks:ks + KT],
                                    qrot[hb:hb + 64, dt, qs + off:qs + SBLK],
                                    start=True, stop=True)
                            pump()
                            pt = [pT_p.tile([128, SBLK], BF16, tag="pt",
                                            name=f"pt{h_}") for h_ in range(2)]
                            for h in range(2):
                                nc.scalar.activation(
                                    out=pt[h][:, off:], in_=sc[h][:, off:],
                                    func=mybir.ActivationFunctionType.Exp,
                                    scale=SCALE)
                            if m >= 0:
                                for h in range(2):
                                    nc.vector.tensor_mul(
                                        pt[h][:, off:], pt[h][:, off:],
                                        maskc[:, m, off:])
                            for h in range(2):
                                nc.tensor.matmul(
                                    ctx[h * 64:h * 64 + 64, off:],
                                    v_sb[:, kt, j * 64:j * 64 + 64],
                                    pt[h][:, off:],
                                    start=first, stop=last,
                                    skip_group_check=True)
                            for h in range(2):
                                nc.tensor.matmul(
                                    sums[h * 64:h * 64 + 64, off:],
                                    ones16[:],
                                    pt[h][:, off:],
                                    start=first, stop=last,
                                    skip_group_check=True)
                            pump()
                        recip = nrm_p.tile([128, SBLK], F32, tag="recip")
                        anrm = nrm_p.tile([128, SBLK], BF16, tag="anrm")
                        if qi < N_SB - 1:
                            nc.vector.reciprocal(recip[:], sums[:])
                            nc.vector.tensor_mul(anrm[:], ctx[:], recip[:])
                            nc.sync.dma_start(out=ag_in[qi][dt, :, :],
                                              in_=anrm[:])
                        else:
                            # split the normalize + gather into qs-halves so
                            # the tail AllGather pipeline is finer-grained
                            HB = SBLK // 2
                            for h2 in range(2):
                                sl = slice(h2 * HB, (h2 + 1) * HB)
                                nc.vector.reciprocal(recip[:, sl], sums[:, sl])
                                nc.vector.tensor_mul(anrm[:, sl], ctx[:, sl],
                                                     recip[:, sl])
                                nc.sync.dma_start(out=ag3_in[dt][h2][:],
                                                  in_=anrm[:, sl])
                                if tp > 1 and not fake_gather:
                                    nc.gpsimd.collective_compute(
                                        "AllGather", mybir.AluOpType.bypass,
                                        replica_groups=tp_groups,
                                        ins=[ag3_in[dt][h2][:].opt()],
                                        outs=[ag3_out[dt][h2][:].opt()])

                # drain remaining fillers
                for op in it:
                    op()

            # ================= main emission =================
            load_x_block(0, split_first=True)
            load_x_block(1)

            # block 0: hc-outer QKV (startup-optimal; psS holds the 4 q banks)
            psq_t = [psS.tile([128, SBLK], F32, tag="pS", name=f"psq{d}")
                     for d in range(N_DT)]
            psk_t = psP.tile([128, SBLK], F32, tag="pP")
            psv_t = psP.tile([128, SBLK], F32, tag="pP")
            for hc in range(N_HT // 4):
                xt = xt_tiles[(0, hc)]
                for hg in range(4):
                    ht = 4 * hc + hg
                    st0, st1 = ht == 0, ht == N_HT - 1
                    for d in range(N_DT):
                        nc.tensor.matmul(psq_t[d][:],
                                         wq_sb[:, ht, d * 128:(d + 1) * 128],
                                         xt[:, hg, :], start=st0, stop=st1,
                                         skip_group_check=True)
                    nc.tensor.matmul(psk_t[:], wk_sb[:, ht, :], xt[:, hg, :],
                                     start=st0, stop=st1, skip_group_check=True)
                    nc.tensor.matmul(psv_t[:], wv_sb[:, ht, :], xt[:, hg, :],
                                     start=st0, stop=st1, skip_group_check=True)
            q_eps = [q_epilogue_ops(0, d, psq_t[d]) for d in range(N_DT)]
            k_eps = k_epilogue_ops(0, psk_t)
            v_eps = v_epilogue_ops(0, psv_t)
            k_eps[0](); v_eps[0]()
            for d in range(N_DT):
                q_eps[d][0]()
            for op in k_eps[1:]:
                op()
            for op in q_eps[0][1:]:
                op()
            for op in v_eps[1:]:
                op()
            for d in range(1, N_DT):
                for op in q_eps[d][1:]:
                    op()

            # blocks 1..3: attention(qi) with QKV(qi+1) as filler
            for qi in range(N_SB - 1):
                load_x_block(qi + 2) if qi + 2 < N_SB else None
                emit_attention(qi, qkv_fillers(qi + 1),
                               pre_pump=(22 if qi == 0 else 0))
                # per-block AllGather right after the block's anrm stores
                if tp > 1 and not fake_gather:
                    nc.gpsimd.collective_compute(
                        "AllGather", mybir.AluOpType.bypass,
                        replica_groups=tp_groups,
                        ins=[ag_in[qi][:].opt()], outs=[ag_out[qi][:].opt()])

            # last block: attention(b3) with o_proj(b0..b2) as filler
            # (per-dt AllGather of b3 emitted inside emit_attention)
            op_fill = []
            dmas = []
            for bk_ in range(N_SB - 1):
                gat = (ag_out[bk_][:].rearrange("r d p s -> (r d) p s")
                       if tp > 1 else ag_in[bk_][:])
                dma_c, mm_ops = oproj_fillers(bk_, gat)
                dmas.append(dma_c)
                op_fill += mm_ops
            # prefetch afull(b0), afull(b1) up front (afl ring = 2); afull(b2)
            # issued a third of the way in (slot freed by o_proj(b0) finish)
            third = len(op_fill) // 3
            op_fill = ([dmas[0], dmas[1]] + op_fill[:third] + [dmas[2]]
                       + op_fill[third:])
            af3 = [[af3_p.tile([128, tp, SBLK // 2], BF16,
                               tag=f"af3_{d}_{h2}", name=f"af3_{d}_{h2}")
                    for h2 in range(2)] for d in range(N_DT)]

            emit_attention(N_SB - 1, op_fill, pump_strips=NKV_L * 2 - 1)
            # af3 loads on the (now idle) sync queue: d0-d2 issue immediately,
            # d3 waits only its own AG; the d3 AG triggers on gpsimd are not
            # delayed by any load
            for d in range(N_DT):
                for h2 in range(2):
                    nc.sync.dma_start(
                        out=af3[d][h2][:],
                        in_=ag3_out[d][h2][:].rearrange("r p s -> p r s"))

            # tail: o_proj of block 3, d-major accumulation
            for sti_l in range(SBLK // 128):
                sti = (N_SB - 1) * (SBLK // 128) + sti_l
                h2 = sti_l // 2
                co = (sti_l % 2) * 128
                pso = psP.tile([128, EO], F32, tag="pP")
                n = 0
                for d in range(N_DT):
                    for r in range(tp):
                        ft = 4 * r + d
                        nc.tensor.matmul(
                            pso[:],
                            af3[d][h2][:, r, co:co + 128],
                            woT_sb[:, ft, :],
                            start=(n == 0), stop=(n == N_FT - 1),
                            skip_group_check=True)
                        n += 1
                ot = osb_p.tile([128, EO], F32, tag="ot", name="ot3")
                nc.scalar.activation(out=ot[:], in_=pso[:],
                                     func=mybir.ActivationFunctionType.Copy,
                                     scale=1.0)
                nc.sync.dma_start(out=out[sti * 128:(sti + 1) * 128, :],
                                  in_=ot[:])

    return inp, out


def make_core_inputs(x, freqs_cos, freqs_sin, Wq, bq, Wk, bk, Wv, bv, Wo,
                     core, s=S, tp=TP):
    """Host-side shard/layout prep for one core (bf16 data, f32 tables)."""
    b, r = core // tp, core % tp
    qh0 = r * NH_L
    kh0 = r * NKV_L
    # head-dim pair interleave: new pos 2u <- old u, 2u+1 <- old u+32.  Scores
    # are invariant (same permutation on q and k); makes rotate-half an
    # adjacent-pair swap (DVE stream_shuffle) instead of a 32-block swap.
    idx = np.empty(HD, np.int64)
    idx[0::2] = np.arange(HD // 2)
    idx[1::2] = np.arange(HD // 2) + HD // 2
    bf16 = ml_dtypes.bfloat16
    xT = np.ascontiguousarray(x[b][:s].T).astype(bf16)
    def permh(W, nh):
        W = W.reshape(nh, HD, -1)[:, idx, :]
        return W.reshape(nh * HD, -1)
    wq_l = permh(Wq[qh0 * HD:(qh0 + NH_L) * HD, :], NH_L)
    wk_l = permh(Wk[kh0 * HD:(kh0 + NKV_L) * HD, :], NKV_L)
    wqT = np.ascontiguousarray(wq_l.T).astype(bf16)
    wkT = np.ascontiguousarray(wk_l.T).astype(bf16)
    wvT = np.ascontiguousarray(
        Wv[kh0 * HD:(kh0 + NKV_L) * HD, :].T).astype(bf16)
    bq_p = bq[qh0 * HD:(qh0 + NH_L) * HD].reshape(NH_L, HD)[:, idx].reshape(-1)
    bk_p = bk[kh0 * HD:(kh0 + NKV_L) * HD].reshape(NKV_L, HD)[:, idx].reshape(-1)
    bq_l = bq_p.reshape(NH_L // 2, 128).T
    bk_l = bk_p.reshape(1, 128).T
    bv_l = bv[kh0 * HD:(kh0 + NKV_L) * HD].reshape(1, 128).T
    # interleaved tables: cos_rep[p] = cos[:, (p%64)//2];
    # sin_pm[p] = (-1 if p even else +1) * sin[:, (p%64)//2]
    u = (np.arange(128) % HD) // 2
    cos_rep = freqs_cos[:s].T[u, :]
    sgn = np.where(np.arange(128) % 2 == 0, -1.0, 1.0)
    sin_pm = freqs_sin[:s].T[u, :] * sgn[:, None]
    woT = np.ascontiguousarray(Wo[r * DQ:(r + 1) * DQ, :].T).astype(bf16)
    return {
        "xT": xT, "wqT": wqT, "wkT": wkT, "wvT": wvT,
        "bq": np.ascontiguousarray(bq_l, np.float32),
        "bk": np.ascontiguousarray(bk_l, np.float32),
        "bv": np.ascontiguousarray(bv_l, np.float32),
        "cos_rep": np.ascontiguousarray(cos_rep, np.float32),
        "sin_pm": np.ascontiguousarray(sin_pm, np.float32),
        "woT": woT,
    }


_CACHED_NC = None


def _get_nc():
    global _CACHED_NC
    if _CACHED_NC is None:
        nc = bacc.Bacc("TRN2", target_bir_lowering=False, debug=False,
                       num_devices=N_CORES)
        build_program(nc)
        nc.compile()
        _CACHED_NC = nc
    return _CACHED_NC


def kernel(x, freqs_cos, freqs_sin, mask, Wq, bq, Wk, bk, Wv, bv, Wo):
    x = np.asarray(x, np.float32)
    args = tuple(np.asarray(a, np.float32) for a in
                 (freqs_cos, freqs_sin, Wq, bq, Wk, bk, Wv, bv, Wo))
    nc = _get_nc()
    in_maps = [make_core_inputs(x, *args, core=c) for c in range(N_CORES)]
    res = bass_utils.run_bass_kernel_spmd(nc, in_maps, core_ids=list(range(N_CORES)))
    out = np.empty((B, S, H), np.float32)
    for c in range(N_CORES):
        b, r = c // TP, c % TP
        out[b][:, r * DQ:(r + 1) * DQ] = res.results[c]["out"]
    return out

